# revision 22
# baseline (speedup 1.0000x reference)
"""Causal self-attention (B=2, T=2048, D=2048, H=16) on 8 TRN2 NeuronCores.

Sharding: tensor-parallel over head pairs (core c owns heads 2c, 2c+1), both
batches processed per core. Each core computes a partial output (its heads'
contribution through the output projection) in bf16; the host sums the 8
partials in fp32.

Device kernel (per core, SPMD), all matmul operands bf16 (fp32 PSUM):
  P1  QKV projection from a host-packed x layout ([128 d, (k,t)] 2MB chunks,
      split into 512KB DMAs). Q^T/K^T produced in [hd, t] rope-split layout;
      rope applied from PSUM with sign-baked sin (4 DVE ops per 128x512
      block). V kept natural [t, f] per 128-t-block.
  P2  Causal attention in S^T layout, the two heads' streams interleaved:
      S^T = K_blk @ Q^T (PSUM, causally narrowed on diagonal blocks),
      E = exp on ACT -> bf16, diagonal masking on DVE, O^T += V_blk.T @ E
      accumulated in PSUM (attn^T layout directly - no transposes), softmax
      denominator via a ones-column matmul into a [1,512] PSUM row, DVE
      reciprocal, GPSIMD row-broadcast, one DVE multiply normalizes into aT.
  P3  Output projection from aT with per-head w_proj slices; PSUM evacuated
      alternately via ACT/DVE into a [128, 2048] bf16 row, one 512KB DMA
      (on the ACT DGE ring) per 128 rows.

_build_nc(loop=k) wraps the whole body in a device-side For_i so test.py can
measure marginal per-execution time; the graded kernel() path uses loop=1
(no loop constructs at all).
"""
import os
import numpy as np
import ml_dtypes

import concourse.bass as bass
from concourse import bacc
import concourse.tile as tile
from concourse import mybir
from concourse import library_config
from concourse.bass_utils import run_bass_kernel_spmd

B, T, D, H = 2, 2048, 2048, 16
HD = D // H            # 128
HPC = H // 8           # heads per core = 2
FL = HPC * HD          # local features = 256
QC = 512               # q-chunk width (and P1 t-chunk width)
NQC = T // QC          # 4
SCALE = float(1.0 / np.sqrt(np.float32(HD)))

f32 = mybir.dt.float32
bf16 = mybir.dt.bfloat16
BF = ml_dtypes.bfloat16

_BUILT = {}


def _build_nc(loop: int = 1):
    dbg = os.environ.get("BASSDBG") == "1"
    nc = bacc.Bacc()

    xbig = nc.dram_tensor("xbig", (B, NQC, 128, 16 * QC), bf16,
                          kind="ExternalInput")
    wqk = nc.dram_tensor("wqk", (128, 16 * 512), bf16, kind="ExternalInput")
    wv = nc.dram_tensor("wv", (128, 16 * 256), bf16, kind="ExternalInput")
    wp = nc.dram_tensor("wp", (128, 2 * 2048), bf16, kind="ExternalInput")
    cs = nc.dram_tensor("cs", (128, 2 * 2048), bf16, kind="ExternalInput")
    msk = nc.dram_tensor("msk", (128, 4 * 512), bf16, kind="ExternalInput")
    onesd = nc.dram_tensor("onesd", (128, 8), bf16, kind="ExternalInput")
    outp = nc.dram_tensor("outp", (B, T, D), bf16, kind="ExternalOutput")
    tick = nc.dram_tensor("tick", (128, 8), f32, kind="ExternalInput")
    tock = nc.dram_tensor("tock", (128, 8), f32, kind="ExternalOutput")
    if dbg:
        dbg_qT = nc.dram_tensor("dbg_qT", (HPC, NQC, 128, QC), bf16,
                                kind="ExternalOutput")
        dbg_kT = nc.dram_tensor("dbg_kT", (HPC, NQC, 128, QC), bf16,
                                kind="ExternalOutput")
        dbg_vP = nc.dram_tensor("dbg_vP", (T // 128, 128, FL), bf16,
                                kind="ExternalOutput")
        dbg_aT = nc.dram_tensor("dbg_aT", (HPC, NQC, 128, QC), bf16,
                                kind="ExternalOutput")
        dbg_lv = nc.dram_tensor("dbg_lv", (HPC, NQC, 128, QC), f32,
                                kind="ExternalOutput")

    nc.gpsimd.load_library(library_config.attnmlp)

    from contextlib import ExitStack
    with tile.TileContext(nc) as tc, ExitStack() as top:
        pers = top.enter_context(tc.tile_pool(name="pers", bufs=1))
        xp = top.enter_context(tc.tile_pool(name="xp", bufs=3))
        rp = top.enter_context(tc.tile_pool(name="rp", bufs=3))
        etp = top.enter_context(tc.tile_pool(name="etp", bufs=6))
        lp = top.enter_context(tc.tile_pool(name="lp", bufs=3))
        op = top.enter_context(tc.tile_pool(name="op", bufs=2))
        ps_qk = top.enter_context(
            tc.tile_pool(name="psqk", bufs=2, space="PSUM"))
        ps_sg = top.enter_context(
            tc.tile_pool(name="pssg", bufs=2, space="PSUM"))
        ps_o = top.enter_context(
            tc.tile_pool(name="pso", bufs=2, space="PSUM"))
        ps_l = top.enter_context(
            tc.tile_pool(name="psl", bufs=2, space="PSUM"))

        def body():
            # persistent constants
            wqk_sb = pers.tile([128, 16 * 512], bf16, tag="wqk")
            wv_sb = pers.tile([128, 16 * 256], bf16, tag="wv")
            wp_sb = pers.tile([128, 2 * 2048], bf16, tag="wp")
            cs_sb = pers.tile([128, 2 * 2048], bf16, tag="cs")
            msk_sb = pers.tile([128, 4 * 512], bf16, tag="msk")
            ones_sb = pers.tile([128, 8], bf16, tag="ones")
            nc.sync.dma_start(out=wqk_sb, in_=wqk[:, :])

            qTc = [[[pers.tile([128, QC], bf16, name=f"qT{b}_{h}_{c}",
                               tag=f"qT{b}_{h}_{c}")
                     for c in range(NQC)] for h in range(HPC)]
                   for b in range(B)]
            kTc = [[[pers.tile([128, QC], bf16, name=f"kT{b}_{h}_{c}",
                               tag=f"kT{b}_{h}_{c}")
                     for c in range(NQC)] for h in range(HPC)]
                   for b in range(B)]
            vP = [[pers.tile([128, FL], bf16, name=f"vP{b}_{i}",
                             tag=f"vP{b}_{i}")
                   for i in range(T // 128)] for b in range(B)]
            aTc = [[[pers.tile([128, QC], bf16, name=f"aT{b}_{h}_{c}",
                               tag=f"aT{b}_{h}_{c}")
                     for c in range(NQC)] for h in range(HPC)]
                   for b in range(B)]

            for b in range(B):
                # ---- P1: QKV + rope ----
                for c in range(NQC):
                    xb = xp.tile([128, 16 * QC], bf16, tag="xb")
                    for g in range(4):
                        nc.sync.dma_start(
                            out=xb[:, g * 4 * QC:(g + 1) * 4 * QC],
                            in_=xbig[b, c, :, g * 4 * QC:(g + 1) * 4 * QC])
                    if b == 0 and c == 0:
                        # non-critical constants after the first x chunk
                        nc.sync.dma_start(out=cs_sb, in_=cs[:, :])
                        nc.sync.dma_start(out=wv_sb, in_=wv[:, :])
                        nc.sync.dma_start(out=msk_sb, in_=msk[:, :])
                        nc.sync.dma_start(out=wp_sb, in_=wp[:, :])
                        nc.sync.dma_start(out=ones_sb, in_=onesd[:, :])
                        tick_sb = pers.tile([128, 8], f32, tag="tick")
                        nc.sync.dma_start(out=tick_sb, in_=tick[:, :])
                        nc.sync.dma_start(out=tock[:, :], in_=tick_sb)
                    for fb in range(4):
                        ps = ps_qk.tile([128, QC], f32, tag="qk")
                        for k in range(16):
                            nc.tensor.matmul(
                                ps[:, :],
                                wqk_sb[:, k * 512 + fb * 128:
                                       k * 512 + (fb + 1) * 128],
                                xb[:, k * QC:(k + 1) * QC],
                                start=(k == 0), stop=(k == 15))
                        dst = (qTc[b][fb][c] if fb < HPC
                               else kTc[b][fb - HPC][c])
                        ca = cs_sb[:, c * QC:(c + 1) * QC]
                        sa_t = cs_sb[0:64, 2048 + c * QC:2048 + (c + 1) * QC]
                        sa_b = cs_sb[64:128, 2048 + c * QC:2048 + (c + 1) * QC]
                        ta = rp.tile([128, QC], f32, tag="ta")
                        t1 = rp.tile([128, QC], f32, tag="tb")
                        nc.vector.tensor_mul(ta, ps[:, :], ca)
                        nc.vector.tensor_mul(t1[0:64, :], ps[64:128, :], sa_t)
                        nc.vector.tensor_mul(t1[64:128, :], ps[0:64, :], sa_b)
                        nc.vector.tensor_add(dst[:, :], ta, t1)
                        if dbg and b == 0:
                            if fb < HPC:
                                nc.sync.dma_start(out=dbg_qT[fb, c], in_=dst)
                            else:
                                nc.sync.dma_start(out=dbg_kT[fb - HPC, c],
                                                  in_=dst)
                    for tb in range(4):
                        ps = ps_qk.tile([128, FL], f32, tag="qk")
                        for k in range(16):
                            nc.tensor.matmul(
                                ps[:, :],
                                xb[:, k * QC + tb * 128:
                                   k * QC + (tb + 1) * 128],
                                wv_sb[:, k * FL:(k + 1) * FL],
                                start=(k == 0), stop=(k == 15))
                        nc.vector.tensor_copy(vP[b][c * 4 + tb], ps[:, :])
                        if dbg and b == 0:
                            nc.sync.dma_start(out=dbg_vP[c * 4 + tb],
                                              in_=vP[b][c * 4 + tb])

                # ---- P2 + P3, per q-chunk (both heads interleaved) ----
                for qc in range(NQC):
                    nsb = 4 * (qc + 1)
                    O = [ps_o.tile([128, QC], f32, tag="o",
                                   name=f"O{b}{qc}{h}")
                         for h in range(HPC)]
                    lps = [ps_l.tile([1, QC], f32, tag="l",
                                     name=f"l{b}{qc}{h}")
                           for h in range(HPC)]
                    for sb in range(nsb):
                        # causal width: diag block d only needs q >= d*128
                        d = sb - qc * 4
                        q0 = d * 128 if d > 0 else 0
                        for h in range(HPC):
                            sg = ps_sg.tile([128, QC], f32, tag="sg")
                            nc.tensor.matmul(
                                sg[:, q0:QC],
                                kTc[b][h][sb // 4][:, (sb % 4) * 128:
                                                   (sb % 4 + 1) * 128],
                                qTc[b][h][qc][:, q0:QC],
                                start=True, stop=True)
                            et = etp.tile([128, QC], bf16, tag="et")
                            nc.scalar.activation(
                                et[:, q0:QC], sg[:, q0:QC],
                                mybir.ActivationFunctionType.Exp, scale=SCALE)
                            if d >= 0:
                                nc.vector.tensor_mul(
                                    et[:, q0:QC], et[:, q0:QC],
                                    msk_sb[:, d * 512 + q0:(d + 1) * 512])
                            nc.tensor.matmul(
                                O[h][:, q0:QC],
                                vP[b][sb][:, h * 128:(h + 1) * 128],
                                et[:, q0:QC],
                                start=(sb == 0), stop=(sb == nsb - 1))
                            nc.tensor.matmul(
                                lps[h][:, q0:QC], ones_sb[:, 0:1],
                                et[:, q0:QC],
                                start=(sb == 0), stop=(sb == nsb - 1))
                    for h in range(HPC):
                        lirow = lp.tile([1, QC], f32, tag="lirow")
                        nc.vector.reciprocal(lirow, lps[h][:, :])
                        livb = lp.tile([128, QC], f32, tag="livb")
                        nc.gpsimd.partition_broadcast(livb[:, :],
                                                      lirow[0:1, :])
                        nc.vector.tensor_mul(aTc[b][h][qc], O[h][:, :], livb)
                        if dbg and b == 0:
                            nc.sync.dma_start(out=dbg_lv[h, qc], in_=livb)
                            nc.sync.dma_start(out=dbg_aT[h, qc],
                                              in_=aTc[b][h][qc])

                    # ---- P3 for this q-chunk's rows ----
                    for j in range(4):
                        tb = qc * 4 + j
                        orow = op.tile([128, 2048], bf16, tag="orow")
                        for ec in range(4):
                            ps3 = ps_qk.tile([128, 512], f32, tag="qk")
                            nc.tensor.matmul(
                                ps3[:, :],
                                aTc[b][0][qc][:, j * 128:(j + 1) * 128],
                                wp_sb[:, ec * 512:(ec + 1) * 512],
                                start=True, stop=False)
                            nc.tensor.matmul(
                                ps3[:, :],
                                aTc[b][1][qc][:, j * 128:(j + 1) * 128],
                                wp_sb[:, 2048 + ec * 512:
                                      2048 + (ec + 1) * 512],
                                start=False, stop=True)
                            dst = orow[:, ec * 512:(ec + 1) * 512]
                            if (tb + ec) % 2 == 0:
                                nc.scalar.activation(
                                    dst, ps3[:, :],
                                    mybir.ActivationFunctionType.Copy)
                            else:
                                nc.vector.tensor_copy(dst, ps3[:, :])
                        nc.sync.dma_start(
                            out=outp[b, tb * 128:(tb + 1) * 128, :],
                            in_=orow)

        if loop == 1:
            body()
        else:
            with tc.For_i(0, loop, 1):
                body()
    nc.finalize()
    return nc


def _prep_in_maps(x, rope, mask, w_attn, w_proj):
    x = np.asarray(x, dtype=np.float32)
    rope = np.asarray(rope, dtype=np.float32)
    mask = np.asarray(mask)
    w_attn = np.asarray(w_attn, dtype=np.float32)
    w_proj = np.asarray(w_proj, dtype=np.float32)

    # x -> (B, NQC, 128, (k, t)) packed chunks
    xr = x.reshape(B, NQC, QC, 16, 128)            # b, c, t, k, p
    xbig = np.ascontiguousarray(
        xr.transpose(0, 1, 4, 3, 2)).reshape(B, NQC, 128, 16 * QC).astype(BF)

    # rope tables, sign-baked sin
    cosT = rope[:, :, 0].T                          # (64, T)
    sinT = rope[:, :, 1].T
    cs = np.zeros((128, 2 * 2048), dtype=np.float32)
    cs[0:64, 0:2048] = cosT
    cs[64:128, 0:2048] = cosT
    cs[0:64, 2048:4096] = -sinT
    cs[64:128, 2048:4096] = sinT
    cs = cs.astype(BF)

    # diagonal mask tiles: msk[p, d*512+q] = allowed(q >= d*128 + p)
    m512 = np.asarray(mask[0, 0, :512, :512])
    mskp = np.zeros((128, 4 * 512), dtype=np.float32)
    for d in range(4):
        mskp[:, d * 512:(d + 1) * 512] = m512[:, d * 128:d * 128 + 128].T
    mskp = mskp.astype(BF)

    perm = np.concatenate([np.arange(0, HD, 2), np.arange(1, HD, 2)])
    in_maps = []
    for core in range(8):
        heads = [2 * core, 2 * core + 1]
        fblocks = [w_attn[h * HD:(h + 1) * HD][perm] for h in heads] + \
                  [w_attn[D + h * HD:D + (h + 1) * HD][perm] for h in heads]
        wqk_pack = np.zeros((128, 16 * 512), dtype=np.float32)
        for k in range(16):
            for fb in range(4):
                wqk_pack[:, k * 512 + fb * 128:k * 512 + (fb + 1) * 128] = \
                    fblocks[fb][:, k * 128:(k + 1) * 128].T
        vrows = [w_attn[2 * D + h * HD:2 * D + (h + 1) * HD] for h in heads]
        wv_pack = np.zeros((128, 16 * 256), dtype=np.float32)
        for k in range(16):
            for hi in range(2):
                wv_pack[:, k * 256 + hi * 128:k * 256 + (hi + 1) * 128] = \
                    vrows[hi][:, k * 128:(k + 1) * 128].T
        wp_pack = np.zeros((128, 2 * 2048), dtype=np.float32)
        for hi in range(2):
            wp_pack[:, hi * 2048:(hi + 1) * 2048] = \
                w_proj[:, core * FL + hi * 128:core * FL + (hi + 1) * 128].T
        im = {
            "xbig": xbig,
            "wqk": wqk_pack.astype(BF),
            "wv": wv_pack.astype(BF),
            "wp": wp_pack.astype(BF),
            "cs": cs,
            "msk": mskp,
            "onesd": np.ones((128, 8), dtype=np.float32).astype(BF),
            "tick": np.zeros((128, 8), np.float32),
        }
        in_maps.append(im)
    return in_maps


def kernel(x, rope, mask, w_attn, w_proj):
    if "nc" not in _BUILT:
        _BUILT["nc"] = _build_nc()
    nc = _BUILT["nc"]
    in_maps = _prep_in_maps(x, rope, mask, w_attn, w_proj)
    res = run_bass_kernel_spmd(nc, in_maps, core_ids=list(range(8)))
    out = np.zeros((B, T, D), dtype=np.float64)
    for c in range(8):
        out += res.results[c]["outp"].astype(np.float64)
    return out.astype(np.float32)
